# revision 22
# baseline (speedup 1.0000x reference)
"""GQA attention-with-KV-cache kernel for Trainium2, sharded over 8 NeuronCores.

Problem: B=32, Q=16 new tokens, DIM=4096, 32 Q-heads / 8 KV-heads, head_dim=128,
cache len 4096 (16 appended at start_pos=4080), rotary on q/k, causal mask.

Sharding: tensor-parallel over KV heads - core c owns KV head c and Q heads
4c..4c+3. Each core computes its heads' attention plus the partial out @ wo_shard;
the host sums the 8 partial outputs (the TP all-reduce).

Pipeline (all phases overlap; measured ~110-120us/core per iteration, ~4x
faster than a version doing transposes on the PE and ~3.5x faster than the
serial-phase baseline):
 - The q projection (x @ wq + rotary + transpose to qTb) runs up front while
   the kv stream starts; xT arrives in token-chunk-major slabs, wq in 4
   chunks so matmuls start as soon as the first chunk lands.
 - Per batch-pair group (16 groups): scores in 8 PSUM windows of [128,512]
   (two batches packed via tile_position), exp with accumulated row sums
   (Act), then each window's UNNORMALIZED exp(p) goes through the DMA-xbar
   transpose straight into the [seq, q'] layout PV needs. On real TRN2 the
   xbar transpose is much faster than PE identity-matmul transposes.
 - Softmax normalization is applied at the end to the PV output during the
   PSUM->SBUF copy, using a reciprocal row broadcast built with two tiny PE
   matmuls (transpose + ones-broadcast), so no [128,4096] normalize pass
   exists and transposes never wait on the row sums.
 - PV accumulates v^T-chunk @ pT-chunk per batch; wo chunks are spread over
   the groups per a fixed schedule; partial outputs stream out through the
   Pool engine's SWDGE queue so writes never block kv loads on SP.
 - kv cache loads are paired (2 batches, 2MB) and issued 3 groups ahead on
   the SP queue, split in sequence halves so the first score windows of a
   group can start before its full pair has landed.

Host-side prep (input marshalling): shard/cast/transpose weights and cache to
bf16 DMA-friendly layouts, compute the 16 appended k/v rows (x @ wk/wv + rotary,
0.5 GFLOP) and splice them into the cache shards. The device does everything
else: xq projection, q-rotary, scores over the 4096-entry cache, softmax,
p @ v, and the partial out @ wo.
"""
import sys
sys.path.insert(0, "/opt/trn_rl_repo")

import numpy as np
import ml_dtypes
from contextlib import ExitStack

import concourse.bass as bass
import concourse.bacc as bacc
import concourse.tile as tile
import concourse.mybir as mybir

BF16 = ml_dtypes.bfloat16

B, Q, DIM = 32, 16, 4096
NH, NKV, HD = 32, 8, 128
NREP = NH // NKV          # 4 q-heads per kv-head
S = 4096                  # cache length
START = S - Q             # 4080
NT = B * Q                # 512 tokens
P = 128
NCORES = 8
QP = NREP * Q             # 64 = q' rows per batch (4 heads x 16 tokens)
NW = S // 512             # 8 score windows per batch-pair group

_CACHE = {}
WARMUP_MM = 20            # PE pstate warmup junk matmuls
PT_ON_PE = False           # p-transpose on PE (True) vs DMA xbar (False)
MARKS = []                 # (label, instruction-id watermark)


def _build_nc(debug=False):
    """Build the single-core Bass program (same program on all 8 cores; only the
    data differs per core)."""
    nc = bacc.Bacc("TRN2", target_bir_lowering=False, debug=debug, num_devices=NCORES)
    dt = mybir.dt
    Exp = mybir.ActivationFunctionType.Exp

    # ---- DRAM I/O (per-core shard layouts, prepared on host) ----
    wq_d = nc.dram_tensor("wq_sh", (32, P, NREP * HD), dt.bfloat16, kind="ExternalInput")
    xT_d = nc.dram_tensor("xTt", (4, P, 32, P), dt.bfloat16, kind="ExternalInput")   # [tok-chunk][d%128][dk][tok%128]
    wo_d = nc.dram_tensor("wo_sh", (4, P, DIM), dt.bfloat16, kind="ExternalInput")   # [c-chunk][128, 4096]
    kT_d = nc.dram_tensor("kT", (B, P, S), dt.bfloat16, kind="ExternalInput")        # per b: updated keys^T [d, seq]
    v_d = nc.dram_tensor("vp", (B, P, S), dt.bfloat16, kind="ExternalInput")         # per b swizzled: [p][c*128+d] = v[c*128+p, d]
    cosq_d = nc.dram_tensor("cosq4", (P, 4 * HD // 2), dt.float32, kind="ExternalInput")  # q rotary x4 hb, pre-scaled 1/sqrt(HD)
    sinq_d = nc.dram_tensor("sinq4", (P, 4 * HD // 2), dt.float32, kind="ExternalInput")
    maskw_d = nc.dram_tensor("mask_w7", (P, 512), dt.bfloat16, kind="ExternalInput")  # additive mask for seq 3584:4096
    id_d = nc.dram_tensor("ident", (P, P), dt.bfloat16, kind="ExternalInput")
    idf_d = nc.dram_tensor("ident_f32", (P, P), dt.float32, kind="ExternalInput")
    out_d = nc.dram_tensor("out_p", (NT, DIM), dt.float16, kind="ExternalOutput")    # partial output (pre all-reduce)

    # wo work (tcT, od) spread over groups: token-chunk tcT completes at
    # group 4*tcT+3; emit 2 od-chunks per group from then on.
    wo_sched = {}
    for tcT in range(4):
        for j, ods in enumerate(([0, 1], [2, 3], [4, 5], [6, 7])):
            g_at = 4 * tcT + 3 + j
            pairs = [(tcT, od) for od in ods]
            if g_at < 16:
                wo_sched.setdefault(g_at, []).extend(pairs)
            else:
                wo_sched.setdefault(-1, []).extend(pairs)

    with ExitStack() as ctx:
        tc = ctx.enter_context(tile.TileContext(nc))

        # ---------- persistent tiles ----------
        cpool = ctx.enter_context(tc.tile_pool(name="const", bufs=1))
        ident = cpool.tile([P, P], dt.bfloat16, tag="ident")
        identf = cpool.tile([P, P], dt.float32, tag="identf")
        ones1 = cpool.tile([1, P], dt.float32, tag="ones1")
        cosq4 = cpool.tile([P, 4 * HD // 2], dt.float32, tag="cosq4")
        sinq4 = cpool.tile([P, 4 * HD // 2], dt.float32, tag="sinq4")
        mask_w7 = cpool.tile([P, 512], dt.bfloat16, tag="maskw")
        # [d, (b, hb, q)] rotated q^T, bf16 - one tile per token chunk
        qTb_t = [cpool.tile([P, 8 * QP], dt.bfloat16, tag=f"qTb{t}", name=f"qTb{t}")
                 for t in range(4)]
        attnT = cpool.tile([P, 4 * NT], dt.bfloat16, tag="attnT")    # [d, (hb, tok)] attention out^T
        wo_sb = cpool.tile([P, 4 * DIM], dt.bfloat16, tag="wo")      # [c-chunk d, (hb, outdim)]
        wq_sb = cpool.tile([P, 32 * 512], dt.bfloat16, tag="wq")

        def qTb_sl(b):
            return qTb_t[b // 8][:, (b % 8) * QP:(b % 8 + 1) * QP]

        # ---------- working pools ----------
        p1pool = ctx.enter_context(tc.tile_pool(name="ph1", bufs=4))
        p1w = ctx.enter_context(tc.tile_pool(name="ph1w", bufs=1))
        kvpool = ctx.enter_context(tc.tile_pool(name="kv", bufs=3))
        p2pool = ctx.enter_context(tc.tile_pool(name="p2", bufs=3))
        ptpool = ctx.enter_context(tc.tile_pool(name="pt", bufs=2))
        smallp = ctx.enter_context(tc.tile_pool(name="small", bufs=2))
        owpool = ctx.enter_context(tc.tile_pool(name="ow", bufs=3))
        spool = ctx.enter_context(tc.tile_pool(name="spsum", bufs=3 if PT_ON_PE else 4, space="PSUM"))
        tpool = ctx.enter_context(tc.tile_pool(name="tpsum", bufs=2 if PT_ON_PE else 1, space="PSUM"))
        smallps = ctx.enter_context(tc.tile_pool(name="smallps", bufs=2, space="PSUM"))
        rnpool = ctx.enter_context(tc.tile_pool(name="rnpsum", bufs=1, space="PSUM"))

        # ---------- const + weight loads (SP queue head) ----------
        nc.sync.dma_start(ident[:], id_d.ap())
        nc.sync.dma_start(identf[:], idf_d.ap())
        nc.sync.dma_start(cosq4[:], cosq_d.ap())
        nc.sync.dma_start(sinq4[:], sinq_d.ap())
        nc.sync.dma_start(mask_w7[:], maskw_d.ap())
        nc.vector.memset(ones1[:], 1.0)

        kv_tiles = {}

        def emit_kv(g):
            b0 = 2 * g
            ktp = kvpool.tile([P, 2, S], dt.bfloat16, tag="ktp", name=f"ktp{g}", bufs=4)
            vtp = kvpool.tile([P, 2, S], dt.bfloat16, tag="vtp", name=f"vtp{g}", bufs=2)
            H = S // 2
            nc.sync.dma_start(ktp[:, :, 0:H], kT_d.ap()[b0:b0 + 2, :, 0:H].rearrange("b p s -> p b s"))
            nc.sync.dma_start(ktp[:, :, H:S], kT_d.ap()[b0:b0 + 2, :, H:S].rearrange("b p s -> p b s"))
            nc.sync.dma_start(vtp[:, :, 0:H], v_d.ap()[b0:b0 + 2, :, 0:H].rearrange("b p s -> p b s"))
            nc.sync.dma_start(vtp[:, :, H:S], v_d.ap()[b0:b0 + 2, :, H:S].rearrange("b p s -> p b s"))
            kv_tiles[g] = (ktp, vtp)

        slab_tiles = {}

        def phase1_load(t, eng):
            subs = []
            for c in range(4):
                sub = p1pool.tile([P, 8 * P], dt.bfloat16, tag="slab", name=f"slab{t}_{c}")
                eng.dma_start(sub[:].rearrange("p (a b) -> p a b", a=8),
                              xT_d.ap()[t, :, c * 8:(c + 1) * 8])
                subs.append(sub)
            slab_tiles[t] = subs

        def phase1(t):
            """x @ wq + rotary + transpose for token chunk t -> qTb_t[t]."""
            subs = slab_tiles.pop(t)
            pq = spool.tile([P, NREP * HD], dt.float32, tag="s", name=f"pq{t}")
            for dk in range(32):
                nc.tensor.matmul(pq[:], subs[dk // 8][:, (dk % 8) * P:(dk % 8 + 1) * P],
                                 wq_sb[:, dk * 512:(dk + 1) * 512],
                                 start=(dk == 0), stop=(dk == 31))
            # rotary over all 4 head-blocks at once (cos/sin pre-tiled + scaled)
            qrot = p1w.tile([P, NREP * HD], dt.bfloat16, tag="qrot", name=f"qrot{t}")
            e = pq[:, 0:NREP * HD:2]
            o = pq[:, 1:NREP * HD:2]
            t1 = p1w.tile([P, NREP * HD // 2], dt.float32, tag="t1", name=f"t1_{t}")
            t2 = p1w.tile([P, NREP * HD // 2], dt.float32, tag="t2", name=f"t2_{t}")
            nc.vector.tensor_mul(t1[:], e, cosq4[:])
            nc.vector.tensor_mul(t2[:], o, sinq4[:])
            nc.vector.tensor_sub(qrot[:, 0:NREP * HD:2], t1[:], t2[:])
            nc.vector.tensor_mul(t1[:], e, sinq4[:])
            nc.vector.tensor_mul(t2[:], o, cosq4[:])
            nc.vector.tensor_add(qrot[:, 1:NREP * HD:2], t1[:], t2[:])
            # PE transpose [tok, (hb d)] -> [d, tok] per hb, then scatter to qTb
            tp = tpool.tile([P, 512], dt.bfloat16, tag="tp", name=f"qtp{t}")
            for j in range(NREP):
                nc.tensor.transpose(tp[:, j * P:(j + 1) * P], qrot[:, j * P:(j + 1) * P], ident[:])
            dstv = qTb_t[t][:].rearrange("p (b hb q) -> p b hb q", hb=NREP, q=Q)
            for j in range(NREP):
                nc.vector.tensor_copy(
                    dstv[:, :, j, :],
                    tp[:, j * P:(j + 1) * P].rearrange("p (b q) -> p b q", q=Q))

        def emit_wo(tcT, od):
            pw = spool.tile([P, 512], dt.float32, tag="s", name="pw")
            for hb in range(4):
                nc.tensor.matmul(
                    pw[:],
                    attnT[:, hb * NT + tcT * P: hb * NT + (tcT + 1) * P],
                    wo_sb[:, hb * DIM + od * 512: hb * DIM + (od + 1) * 512],
                    start=(hb == 0), stop=(hb == 3))
            ow = owpool.tile([P, 512], dt.float16, tag="ow", name="ow")
            nc.vector.tensor_copy(ow[:], pw[:])
            nc.gpsimd.dma_start(
                out_d.ap()[tcT * P:(tcT + 1) * P, od * 512:(od + 1) * 512], ow[:])

        def group(g):
            MARKS.append((f"g{g}", nc.next_id()))
            b0, b1 = 2 * g, 2 * g + 1
            ktp, vtp = kv_tiles.pop(g)
            pT = ptpool.tile([P, S], dt.bfloat16, tag="pT", name=f"pT{g}")
            accs = smallp.tile([P, 16], dt.float32, tag="accs", name=f"accs{g}")
            rinv = smallp.tile([P, 1], dt.float32, tag="rinv", name=f"rinv{g}")
            rT_sb = smallp.tile([1, P], dt.float32, tag="rTsb", name=f"rT{g}")
            rbc_sb = smallp.tile([P, P], dt.float32, tag="rbc", name=f"rbc{g}")

            # scores -> exp (unnormalized, accum row sums) -> PE transpose -> pT
            for w in range(NW):
                s = spool.tile([P, 512], dt.float32, tag="s", name=f"s{g}_{w}")
                nc.tensor.matmul(s[0:QP, :], qTb_sl(b0), ktp[:, 0, w * 512:(w + 1) * 512],
                                 tile_position=(0, 0))
                nc.tensor.matmul(s[QP:P, :], qTb_sl(b1), ktp[:, 1, w * 512:(w + 1) * 512],
                                 tile_position=(0, QP))
                if w == NW - 1:
                    nc.vector.tensor_add(s[:], s[:], mask_w7[:])
                p2w = p2pool.tile([P, 512], dt.bfloat16, tag="p2", name=f"p2_{g}_{w}")
                nc.scalar.activation(p2w[:], s[:], Exp, accum_out=accs[:, w:w + 1])
                tp = tpool.tile([P, 512], dt.bfloat16, tag="tp", name=f"tp{g}_{w}")
                for j in range(4):
                    nc.tensor.transpose(tp[:, j * P:(j + 1) * P], p2w[:, j * P:(j + 1) * P],
                                        ident[:])
                nc.vector.tensor_copy(pT[:, w * 512:(w + 1) * 512], tp[:])

            # normalizer chain interleaved with pv so no engine stalls on it:
            # reduce/recip (DVE) overlap pv b0; rT/rbc (PE) slot between pv runs.
            nc.vector.reduce_sum(rinv[:], accs[:, 0:NW], axis=mybir.AxisListType.X)
            nc.vector.reciprocal(rinv[:], rinv[:])

            rn = rnpool.tile([P, 2 * P], dt.float32, tag="rn", name=f"rn{g}")
            pos = {}
            for bi, boff in ((0, 0), (1, QP)):
                po = smallps.tile([P, QP], dt.float32, tag="po", name=f"po{g}_{bi}")
                for c in range(S // P):
                    nc.tensor.matmul(po[:], vtp[:, bi, c * P:(c + 1) * P],
                                     pT[:, c * P + boff: c * P + boff + QP],
                                     start=(c == 0), stop=(c == S // P - 1))
                pos[bi] = po
                if bi == 0:
                    nc.tensor.transpose(rn[0:1, 0:P], rinv[:], identf[:])
                    nc.vector.tensor_copy(rT_sb[:], rn[0:1, 0:P])
            nc.tensor.matmul(rn[:, P:2 * P], ones1[:], rT_sb[:])
            nc.vector.tensor_copy(rbc_sb[:], rn[:, P:2 * P])

            # normalize during PSUM->attnT copy
            for bi, boff in ((0, 0), (1, QP)):
                dst = attnT[:].rearrange("p (hb t) -> p hb t", hb=4)[
                    :, :, (b0 + bi) * Q:(b0 + bi + 1) * Q]
                src = pos[bi][:].rearrange("p (hb q) -> p hb q", hb=4)
                rsl = rbc_sb[:, boff:boff + QP].rearrange("p (hb q) -> p hb q", hb=4)
                nc.vector.tensor_mul(dst, src, rsl)

            for tcT, od in wo_sched.get(g, []):
                emit_wo(tcT, od)

        # ---------- emission: interleave phase 1, kv prefetch, groups ----------
        MARKS.clear()
        # PE warmup junk: keep PE busy from t~0 so phase-1 matmuls run at full
        # pstate (cold PE runs 2-4x slower in the first ~3us of a busy burst).
        jk = rnpool.tile([P, 2 * P], dt.float32, tag="rn", name="jk")
        for _ in range(WARMUP_MM):
            nc.tensor.matmul(jk[:, 0:P], ident[:], ident[:])
        # all of phase 1 runs up front: slabs+wq stream first (~23us), the four
        # qTb chunks compute while kv starts; no phase-1 work mid-stream.
        phase1_load(0, nc.sync)
        for c in range(4):
            nc.sync.dma_start(
                wq_sb[:, c * 8 * 512:(c + 1) * 8 * 512].rearrange("p (a b) -> p a b", a=8),
                wq_d.ap()[c * 8:(c + 1) * 8].rearrange("a p b -> p a b"))
        phase1_load(1, nc.sync)
        phase1_load(2, nc.sync)
        phase1_load(3, nc.sync)
        phase1(0)
        emit_kv(0)
        phase1(1)
        emit_kv(1)
        phase1(2)
        emit_kv(2)
        phase1(3)
        nc.sync.dma_start(wo_sb[:].rearrange("p (a b) -> p a b", a=4),
                          wo_d.ap().rearrange("a p b -> p a b"))
        for g in range(B // 2):
            group(g)
            if g + 3 <= 15:
                emit_kv(g + 3)

        for tcT, od in wo_sched.get(-1, []):
            emit_wo(tcT, od)

    nc.compile()
    return nc


def _host_prep(x, cache_k, cache_v, freqs_cis, mask, wq, wk, wv, wo):
    """Build the 8 per-core input maps. Computes the 16 appended k/v rows here
    (cheap projection) and splices them into the cache shards."""
    xf = np.asarray(x, dtype=np.float32).reshape(NT, DIM)
    xbf = xf.astype(BF16).astype(np.float32)      # reference casts x to bf16 first
    xT_full = np.ascontiguousarray(xbf.T)         # (4096 d, 512 tok) fp32
    # device layout [t][d%128][dk][tok%128]
    xTt = np.ascontiguousarray(
        xT_full.reshape(32, P, 4, P).transpose(2, 1, 0, 3)).astype(BF16)

    wq = np.asarray(wq); wk = np.asarray(wk); wv = np.asarray(wv); wo = np.asarray(wo)

    fc = np.asarray(freqs_cis)
    if np.iscomplexobj(fc):
        cos16 = np.real(fc).astype(np.float32)    # (16, 64)
        sin16 = np.imag(fc).astype(np.float32)
    else:
        cos16 = np.cos(fc).astype(np.float32)
        sin16 = np.sin(fc).astype(np.float32)
    scale = np.float32(1.0 / np.sqrt(HD))
    cosq = np.tile(cos16, (8, 1)) * scale         # (128, 64) rows: q = r % 16
    sinq = np.tile(sin16, (8, 1)) * scale
    cosq4 = np.tile(cosq, (1, 4))                 # (128, 256): per head-block
    sinq4 = np.tile(sinq, (1, 4))

    # appended k/v rows (host projection, matches reference numerics closely:
    # bf16-valued operands, fp32 accumulate)
    wkf = wk.astype(np.float32)
    wvf = wv.astype(np.float32)
    xk = (xbf @ wkf).reshape(B, Q, NKV, HD)
    xv = (xbf @ wvf).reshape(B, Q, NKV, HD)
    e = xk[..., 0::2]; o = xk[..., 1::2]
    c4 = cos16[None, :, None, :]; s4 = sin16[None, :, None, :]
    xkr = np.empty_like(xk)
    xkr[..., 0::2] = e * c4 - o * s4
    xkr[..., 1::2] = e * s4 + o * c4

    # full updated cache, then per-core layouts
    ck = np.asarray(cache_k, dtype=np.float32).copy()
    cv = np.asarray(cache_v, dtype=np.float32).copy()
    ck[:, START:S] = xkr
    cv[:, START:S] = xv

    kT_all = np.ascontiguousarray(ck.transpose(2, 0, 3, 1)).astype(BF16)  # (kv, b, d, s)
    v_r = cv.reshape(B, 32, P, NKV, HD)
    v_all = np.ascontiguousarray(v_r.transpose(3, 0, 2, 1, 4)).astype(BF16)  # (kv, b, p, c, d)
    v_all = v_all.reshape(NKV, B, P, S)

    # additive mask for score window 7 (seq 3584:4096): causal on the last 16
    mask_np = np.asarray(mask, dtype=np.float32)
    mask_w7 = np.zeros((P, 512), dtype=np.float32)
    mask_w7[:, 496:512] = np.tile(mask_np[:, START:S], (8, 1))
    mask_w7 = mask_w7.astype(BF16)

    in_maps = []
    for c in range(NCORES):
        hq0 = c * NREP * HD
        in_maps.append({
            "xTt": xTt,
            "wq_sh": np.ascontiguousarray(wq[:, hq0:hq0 + NREP * HD]).astype(BF16).reshape(32, P, NREP * HD),
            "wo_sh": np.ascontiguousarray(wo[hq0:hq0 + NREP * HD, :]).astype(BF16).reshape(4, P, DIM),
            "kT": kT_all[c],
            "vp": v_all[c],
            "cosq4": cosq4, "sinq4": sinq4,
            "mask_w7": mask_w7,
            "ident": np.eye(P, dtype=BF16),
            "ident_f32": np.eye(P, dtype=np.float32),
        })
    return in_maps


def _get_nc():
    if "nc" not in _CACHE:
        _CACHE["nc"] = _build_nc(debug=False)
    return _CACHE["nc"]


def kernel(x, cache_k, cache_v, freqs_cis, mask, wq, wk, wv, wo, start_pos):
    assert int(start_pos) == START, f"kernel hardcodes start_pos={START}"
    from concourse import bass_utils
    nc = _get_nc()
    in_maps = _host_prep(x, cache_k, cache_v, freqs_cis, mask, wq, wk, wv, wo)
    res = bass_utils.run_bass_kernel_spmd(nc, in_maps, core_ids=list(range(NCORES)))
    out = np.zeros((NT, DIM), dtype=np.float32)
    for c in range(NCORES):
        out += np.asarray(res.results[c]["out_p"], dtype=np.float32)
    return out.reshape(B, Q, DIM)


# revision 23
# speedup vs baseline: 1.2091x; 1.2091x over previous
"""GQA attention-with-KV-cache kernel for Trainium2, sharded over 8 NeuronCores.

Problem: B=32, Q=16 new tokens, DIM=4096, 32 Q-heads / 8 KV-heads, head_dim=128,
cache len 4096 (16 appended at start_pos=4080), rotary on q/k, causal mask.

Sharding: tensor-parallel over KV heads - core c owns KV head c and Q heads
4c..4c+3. Each core computes its heads' attention plus the partial out @ wo_shard;
the host sums the 8 partial outputs (the TP all-reduce).

v2 design (vs v1): single fused pipeline tuned to keep the DMA engines (the
roofline resource: ~80 MB/core of HBM traffic) 100% busy from t=0:
 - phase 1 (x @ wq + rotary) is interleaved with the kv cache stream instead
   of running before it; xT arrives in token-chunk-major slabs.
 - p-transposes moved from the DMA xbar to the PE (identity-matmul transpose,
   out-free-size cost only), freeing ~57us of DMA engine time.
 - softmax normalization applied to the PV output (oT) via a PE-broadcast
   reciprocal tile instead of scaling the full [128,4096] p matrix, removing
   the normalize->transpose serialization.
 - kv loads paired (2 MB DMAs), issued alone on the SP queue; output writes
   go through the Pool engine's SWDGE queue so no stream head-of-line blocks
   another.

Host-side prep (input marshalling): shard/cast/transpose weights and cache to
bf16 DMA-friendly layouts, compute the 16 appended k/v rows (x @ wk/wv + rotary,
0.5 GFLOP) and splice them into the cache shards. The device does everything
else: xq projection, q-rotary, scores over the 4096-entry cache, softmax,
p @ v, and the partial out @ wo.
"""
import sys
sys.path.insert(0, "/opt/trn_rl_repo")

import numpy as np
import ml_dtypes
from contextlib import ExitStack

import concourse.bass as bass
import concourse.bacc as bacc
import concourse.tile as tile
import concourse.mybir as mybir

BF16 = ml_dtypes.bfloat16

B, Q, DIM = 32, 16, 4096
NH, NKV, HD = 32, 8, 128
NREP = NH // NKV          # 4 q-heads per kv-head
S = 4096                  # cache length
START = S - Q             # 4080
NT = B * Q                # 512 tokens
P = 128
NCORES = 8
QP = NREP * Q             # 64 = q' rows per batch (4 heads x 16 tokens)
NW = S // 512             # 8 score windows per batch-pair group

_CACHE = {}
WARMUP_MM = 20            # PE pstate warmup junk matmuls
PT_ON_PE = False           # p-transpose on PE (True) vs DMA xbar (False)
MARKS = []                 # (label, instruction-id watermark)


def _build_nc(debug=False):
    """Build the single-core Bass program (same program on all 8 cores; only the
    data differs per core)."""
    nc = bacc.Bacc("TRN2", target_bir_lowering=False, debug=debug, num_devices=NCORES)
    dt = mybir.dt
    Exp = mybir.ActivationFunctionType.Exp

    # ---- DRAM I/O (per-core shard layouts, prepared on host) ----
    wq_d = nc.dram_tensor("wq_sh", (32, P, NREP * HD), dt.bfloat16, kind="ExternalInput")
    xT_d = nc.dram_tensor("xTt", (4, P, 32, P), dt.bfloat16, kind="ExternalInput")   # [tok-chunk][d%128][dk][tok%128]
    wo_d = nc.dram_tensor("wo_sh", (4, P, DIM), dt.bfloat16, kind="ExternalInput")   # [c-chunk][128, 4096]
    kT_d = nc.dram_tensor("kT", (B, P, S), dt.bfloat16, kind="ExternalInput")        # per b: updated keys^T [d, seq]
    v_d = nc.dram_tensor("vp", (B, P, S), dt.bfloat16, kind="ExternalInput")         # per b swizzled: [p][c*128+d] = v[c*128+p, d]
    cosq_d = nc.dram_tensor("cosq4", (P, 4 * HD // 2), dt.float32, kind="ExternalInput")  # q rotary x4 hb, pre-scaled 1/sqrt(HD)
    sinq_d = nc.dram_tensor("sinq4", (P, 4 * HD // 2), dt.float32, kind="ExternalInput")
    maskw_d = nc.dram_tensor("mask_w7", (P, 512), dt.bfloat16, kind="ExternalInput")  # additive mask for seq 3584:4096
    id_d = nc.dram_tensor("ident", (P, P), dt.bfloat16, kind="ExternalInput")
    idf_d = nc.dram_tensor("ident_f32", (P, P), dt.float32, kind="ExternalInput")
    out_d = nc.dram_tensor("out_p", (NT, DIM), dt.float16, kind="ExternalOutput")    # partial output (pre all-reduce)

    # wo work (tcT, od) spread over groups: token-chunk tcT completes at
    # group 4*tcT+3; emit 2 od-chunks per group from then on.
    wo_sched = {}
    for tcT in range(4):
        for j, ods in enumerate(([0, 1], [2, 3], [4, 5], [6, 7])):
            g_at = 4 * tcT + 3 + j
            pairs = [(tcT, od) for od in ods]
            if g_at < 16:
                wo_sched.setdefault(g_at, []).extend(pairs)
            else:
                wo_sched.setdefault(-1, []).extend(pairs)

    with ExitStack() as ctx:
        tc = ctx.enter_context(tile.TileContext(nc))

        # ---------- persistent tiles ----------
        cpool = ctx.enter_context(tc.tile_pool(name="const", bufs=1))
        ident = cpool.tile([P, P], dt.bfloat16, tag="ident")
        identf = cpool.tile([P, P], dt.float32, tag="identf")
        ones1 = cpool.tile([1, P], dt.float32, tag="ones1")
        cosq4 = cpool.tile([P, 4 * HD // 2], dt.float32, tag="cosq4")
        sinq4 = cpool.tile([P, 4 * HD // 2], dt.float32, tag="sinq4")
        mask_w7 = cpool.tile([P, 512], dt.bfloat16, tag="maskw")
        # [d, (b, hb, q)] rotated q^T, bf16 - one tile per token chunk
        qTb_t = [cpool.tile([P, 8 * QP], dt.bfloat16, tag=f"qTb{t}", name=f"qTb{t}")
                 for t in range(4)]
        attnT = cpool.tile([P, 4 * NT], dt.bfloat16, tag="attnT")    # [d, (hb, tok)] attention out^T
        wo_sb = cpool.tile([P, 4 * DIM], dt.bfloat16, tag="wo")      # [c-chunk d, (hb, outdim)]
        wq_sb = cpool.tile([P, 32 * 512], dt.bfloat16, tag="wq")

        def qTb_sl(b):
            return qTb_t[b // 8][:, (b % 8) * QP:(b % 8 + 1) * QP]

        # ---------- working pools ----------
        p1pool = ctx.enter_context(tc.tile_pool(name="ph1", bufs=4))
        p1w = ctx.enter_context(tc.tile_pool(name="ph1w", bufs=1))
        kvpool = ctx.enter_context(tc.tile_pool(name="kv", bufs=3))
        p2pool = ctx.enter_context(tc.tile_pool(name="p2", bufs=3))
        ptpool = ctx.enter_context(tc.tile_pool(name="pt", bufs=2))
        smallp = ctx.enter_context(tc.tile_pool(name="small", bufs=2))
        owpool = ctx.enter_context(tc.tile_pool(name="ow", bufs=3))
        spool = ctx.enter_context(tc.tile_pool(name="spsum", bufs=3 if PT_ON_PE else 4, space="PSUM"))
        tpool = ctx.enter_context(tc.tile_pool(name="tpsum", bufs=2 if PT_ON_PE else 1, space="PSUM"))
        smallps = ctx.enter_context(tc.tile_pool(name="smallps", bufs=2, space="PSUM"))
        rnpool = ctx.enter_context(tc.tile_pool(name="rnpsum", bufs=1, space="PSUM"))

        # ---------- const + weight loads (SP queue head) ----------
        nc.sync.dma_start(ident[:], id_d.ap())
        nc.sync.dma_start(identf[:], idf_d.ap())
        nc.sync.dma_start(cosq4[:], cosq_d.ap())
        nc.sync.dma_start(sinq4[:], sinq_d.ap())
        nc.sync.dma_start(mask_w7[:], maskw_d.ap())
        nc.vector.memset(ones1[:], 1.0)

        kv_tiles = {}

        def emit_kv(g):
            b0 = 2 * g
            ktp = kvpool.tile([P, 2, S], dt.bfloat16, tag="ktp", name=f"ktp{g}")
            vtp = kvpool.tile([P, 2, S], dt.bfloat16, tag="vtp", name=f"vtp{g}")
            H = S // 2
            nc.sync.dma_start(ktp[:, :, 0:H], kT_d.ap()[b0:b0 + 2, :, 0:H].rearrange("b p s -> p b s"))
            nc.sync.dma_start(ktp[:, :, H:S], kT_d.ap()[b0:b0 + 2, :, H:S].rearrange("b p s -> p b s"))
            nc.sync.dma_start(vtp[:, :, 0:H], v_d.ap()[b0:b0 + 2, :, 0:H].rearrange("b p s -> p b s"))
            nc.sync.dma_start(vtp[:, :, H:S], v_d.ap()[b0:b0 + 2, :, H:S].rearrange("b p s -> p b s"))
            kv_tiles[g] = (ktp, vtp)

        slab_tiles = {}

        def phase1_load(t, eng):
            subs = []
            for c in range(4):
                sub = p1pool.tile([P, 8 * P], dt.bfloat16, tag="slab", name=f"slab{t}_{c}")
                eng.dma_start(sub[:].rearrange("p (a b) -> p a b", a=8),
                              xT_d.ap()[t, :, c * 8:(c + 1) * 8])
                subs.append(sub)
            slab_tiles[t] = subs

        def phase1(t):
            """x @ wq + rotary + transpose for token chunk t -> qTb_t[t]."""
            subs = slab_tiles.pop(t)
            pq = spool.tile([P, NREP * HD], dt.float32, tag="s", name=f"pq{t}")
            for dk in range(32):
                nc.tensor.matmul(pq[:], subs[dk // 8][:, (dk % 8) * P:(dk % 8 + 1) * P],
                                 wq_sb[:, dk * 512:(dk + 1) * 512],
                                 start=(dk == 0), stop=(dk == 31))
            # rotary over all 4 head-blocks at once (cos/sin pre-tiled + scaled)
            qrot = p1w.tile([P, NREP * HD], dt.bfloat16, tag="qrot", name=f"qrot{t}")
            e = pq[:, 0:NREP * HD:2]
            o = pq[:, 1:NREP * HD:2]
            t1 = p1w.tile([P, NREP * HD // 2], dt.float32, tag="t1", name=f"t1_{t}")
            t2 = p1w.tile([P, NREP * HD // 2], dt.float32, tag="t2", name=f"t2_{t}")
            nc.vector.tensor_mul(t1[:], e, cosq4[:])
            nc.vector.tensor_mul(t2[:], o, sinq4[:])
            nc.vector.tensor_sub(qrot[:, 0:NREP * HD:2], t1[:], t2[:])
            nc.vector.tensor_mul(t1[:], e, sinq4[:])
            nc.vector.tensor_mul(t2[:], o, cosq4[:])
            nc.vector.tensor_add(qrot[:, 1:NREP * HD:2], t1[:], t2[:])
            # PE transpose [tok, (hb d)] -> [d, tok] per hb, then scatter to qTb
            tp = tpool.tile([P, 512], dt.bfloat16, tag="tp", name=f"qtp{t}")
            for j in range(NREP):
                nc.tensor.transpose(tp[:, j * P:(j + 1) * P], qrot[:, j * P:(j + 1) * P], ident[:])
            dstv = qTb_t[t][:].rearrange("p (b hb q) -> p b hb q", hb=NREP, q=Q)
            for j in range(NREP):
                nc.vector.tensor_copy(
                    dstv[:, :, j, :],
                    tp[:, j * P:(j + 1) * P].rearrange("p (b q) -> p b q", q=Q))

        def emit_wo(tcT, od):
            pw = spool.tile([P, 512], dt.float32, tag="s", name="pw")
            for hb in range(4):
                nc.tensor.matmul(
                    pw[:],
                    attnT[:, hb * NT + tcT * P: hb * NT + (tcT + 1) * P],
                    wo_sb[:, hb * DIM + od * 512: hb * DIM + (od + 1) * 512],
                    start=(hb == 0), stop=(hb == 3))
            ow = owpool.tile([P, 512], dt.float16, tag="ow", name="ow")
            nc.scalar.copy(ow[:], pw[:])
            nc.gpsimd.dma_start(
                out_d.ap()[tcT * P:(tcT + 1) * P, od * 512:(od + 1) * 512], ow[:])

        def group(g):
            MARKS.append((f"g{g}", nc.next_id()))
            b0, b1 = 2 * g, 2 * g + 1
            ktp, vtp = kv_tiles.pop(g)
            pT = ptpool.tile([P, S], dt.bfloat16, tag="pT", name=f"pT{g}")
            accs = smallp.tile([P, 16], dt.float32, tag="accs", name=f"accs{g}")
            rinv = smallp.tile([P, 1], dt.float32, tag="rinv", name=f"rinv{g}")
            rT_sb = smallp.tile([1, P], dt.float32, tag="rTsb", name=f"rT{g}")
            rbc_sb = smallp.tile([P, P], dt.float32, tag="rbc", name=f"rbc{g}")

            # scores -> exp (unnormalized, accum row sums) -> PE transpose -> pT
            for w in range(NW):
                s = spool.tile([P, 512], dt.float32, tag="s", name=f"s{g}_{w}")
                nc.tensor.matmul(s[0:QP, :], qTb_sl(b0), ktp[:, 0, w * 512:(w + 1) * 512],
                                 tile_position=(0, 0))
                nc.tensor.matmul(s[QP:P, :], qTb_sl(b1), ktp[:, 1, w * 512:(w + 1) * 512],
                                 tile_position=(0, QP))
                if w == NW - 1:
                    nc.vector.tensor_add(s[:], s[:], mask_w7[:])
                p2w = p2pool.tile([P, 512], dt.bfloat16, tag="p2", name=f"p2_{g}_{w}")
                nc.scalar.activation(p2w[:], s[:], Exp, accum_out=accs[:, w:w + 1])
                tp = tpool.tile([P, 512], dt.bfloat16, tag="tp", name=f"tp{g}_{w}")
                for j in range(4):
                    nc.tensor.transpose(tp[:, j * P:(j + 1) * P], p2w[:, j * P:(j + 1) * P],
                                        ident[:])
                nc.vector.tensor_copy(pT[:, w * 512:(w + 1) * 512], tp[:])

            # normalizer chain interleaved with pv so no engine stalls on it:
            # reduce/recip (DVE) overlap pv b0; rT/rbc (PE) slot between pv runs.
            nc.vector.reduce_sum(rinv[:], accs[:, 0:NW], axis=mybir.AxisListType.X)
            nc.vector.reciprocal(rinv[:], rinv[:])

            rn = rnpool.tile([P, 2 * P], dt.float32, tag="rn", name=f"rn{g}")
            pos = {}
            for bi, boff in ((0, 0), (1, QP)):
                po = smallps.tile([P, QP], dt.float32, tag="po", name=f"po{g}_{bi}")
                for c in range(S // P):
                    nc.tensor.matmul(po[:], vtp[:, bi, c * P:(c + 1) * P],
                                     pT[:, c * P + boff: c * P + boff + QP],
                                     start=(c == 0), stop=(c == S // P - 1))
                pos[bi] = po
                if bi == 0:
                    nc.tensor.transpose(rn[0:1, 0:P], rinv[:], identf[:])
                    nc.vector.tensor_copy(rT_sb[:], rn[0:1, 0:P])
            nc.tensor.matmul(rn[:, P:2 * P], ones1[:], rT_sb[:])
            nc.vector.tensor_copy(rbc_sb[:], rn[:, P:2 * P])

            # normalize during PSUM->attnT copy
            for bi, boff in ((0, 0), (1, QP)):
                dst = attnT[:].rearrange("p (hb t) -> p hb t", hb=4)[
                    :, :, (b0 + bi) * Q:(b0 + bi + 1) * Q]
                src = pos[bi][:].rearrange("p (hb q) -> p hb q", hb=4)
                rsl = rbc_sb[:, boff:boff + QP].rearrange("p (hb q) -> p hb q", hb=4)
                nc.vector.tensor_mul(dst, src, rsl)

            for tcT, od in wo_sched.get(g, []):
                emit_wo(tcT, od)

        # ---------- emission: interleave phase 1, kv prefetch, groups ----------
        MARKS.clear()
        # PE warmup junk: keep PE busy from t~0 so phase-1 matmuls run at full
        # pstate (cold PE runs 2-4x slower in the first ~3us of a busy burst).
        jk = rnpool.tile([P, 2 * P], dt.float32, tag="rn", name="jk")
        for _ in range(WARMUP_MM):
            nc.tensor.matmul(jk[:, 0:P], ident[:], ident[:])
        # all of phase 1 runs up front: slabs+wq stream first (~23us), the four
        # qTb chunks compute while kv starts; no phase-1 work mid-stream.
        phase1_load(0, nc.sync)
        for c in range(4):
            nc.sync.dma_start(
                wq_sb[:, c * 8 * 512:(c + 1) * 8 * 512].rearrange("p (a b) -> p a b", a=8),
                wq_d.ap()[c * 8:(c + 1) * 8].rearrange("a p b -> p a b"))
        phase1_load(1, nc.sync)
        phase1_load(2, nc.sync)
        phase1_load(3, nc.sync)
        phase1(0)
        emit_kv(0)
        phase1(1)
        emit_kv(1)
        phase1(2)
        emit_kv(2)
        phase1(3)
        nc.sync.dma_start(wo_sb[:].rearrange("p (a b) -> p a b", a=4),
                          wo_d.ap().rearrange("a p b -> p a b"))
        for g in range(B // 2):
            group(g)
            if g + 3 <= 15:
                emit_kv(g + 3)

        for tcT, od in wo_sched.get(-1, []):
            emit_wo(tcT, od)

    nc.compile()
    return nc


def _host_prep(x, cache_k, cache_v, freqs_cis, mask, wq, wk, wv, wo):
    """Build the 8 per-core input maps. Computes the 16 appended k/v rows here
    (cheap projection) and splices them into the cache shards."""
    xf = np.asarray(x, dtype=np.float32).reshape(NT, DIM)
    xbf = xf.astype(BF16).astype(np.float32)      # reference casts x to bf16 first
    xT_full = np.ascontiguousarray(xbf.T)         # (4096 d, 512 tok) fp32
    # device layout [t][d%128][dk][tok%128]
    xTt = np.ascontiguousarray(
        xT_full.reshape(32, P, 4, P).transpose(2, 1, 0, 3)).astype(BF16)

    wq = np.asarray(wq); wk = np.asarray(wk); wv = np.asarray(wv); wo = np.asarray(wo)

    fc = np.asarray(freqs_cis)
    if np.iscomplexobj(fc):
        cos16 = np.real(fc).astype(np.float32)    # (16, 64)
        sin16 = np.imag(fc).astype(np.float32)
    else:
        cos16 = np.cos(fc).astype(np.float32)
        sin16 = np.sin(fc).astype(np.float32)
    scale = np.float32(1.0 / np.sqrt(HD))
    cosq = np.tile(cos16, (8, 1)) * scale         # (128, 64) rows: q = r % 16
    sinq = np.tile(sin16, (8, 1)) * scale
    cosq4 = np.tile(cosq, (1, 4))                 # (128, 256): per head-block
    sinq4 = np.tile(sinq, (1, 4))

    # appended k/v rows (host projection, matches reference numerics closely:
    # bf16-valued operands, fp32 accumulate)
    wkf = wk.astype(np.float32)
    wvf = wv.astype(np.float32)
    xk = (xbf @ wkf).reshape(B, Q, NKV, HD)
    xv = (xbf @ wvf).reshape(B, Q, NKV, HD)
    e = xk[..., 0::2]; o = xk[..., 1::2]
    c4 = cos16[None, :, None, :]; s4 = sin16[None, :, None, :]
    xkr = np.empty_like(xk)
    xkr[..., 0::2] = e * c4 - o * s4
    xkr[..., 1::2] = e * s4 + o * c4

    # full updated cache, then per-core layouts
    ck = np.asarray(cache_k, dtype=np.float32).copy()
    cv = np.asarray(cache_v, dtype=np.float32).copy()
    ck[:, START:S] = xkr
    cv[:, START:S] = xv

    kT_all = np.ascontiguousarray(ck.transpose(2, 0, 3, 1)).astype(BF16)  # (kv, b, d, s)
    v_r = cv.reshape(B, 32, P, NKV, HD)
    v_all = np.ascontiguousarray(v_r.transpose(3, 0, 2, 1, 4)).astype(BF16)  # (kv, b, p, c, d)
    v_all = v_all.reshape(NKV, B, P, S)

    # additive mask for score window 7 (seq 3584:4096): causal on the last 16
    mask_np = np.asarray(mask, dtype=np.float32)
    mask_w7 = np.zeros((P, 512), dtype=np.float32)
    mask_w7[:, 496:512] = np.tile(mask_np[:, START:S], (8, 1))
    mask_w7 = mask_w7.astype(BF16)

    in_maps = []
    for c in range(NCORES):
        hq0 = c * NREP * HD
        in_maps.append({
            "xTt": xTt,
            "wq_sh": np.ascontiguousarray(wq[:, hq0:hq0 + NREP * HD]).astype(BF16).reshape(32, P, NREP * HD),
            "wo_sh": np.ascontiguousarray(wo[hq0:hq0 + NREP * HD, :]).astype(BF16).reshape(4, P, DIM),
            "kT": kT_all[c],
            "vp": v_all[c],
            "cosq4": cosq4, "sinq4": sinq4,
            "mask_w7": mask_w7,
            "ident": np.eye(P, dtype=BF16),
            "ident_f32": np.eye(P, dtype=np.float32),
        })
    return in_maps


def _get_nc():
    if "nc" not in _CACHE:
        _CACHE["nc"] = _build_nc(debug=False)
    return _CACHE["nc"]


def kernel(x, cache_k, cache_v, freqs_cis, mask, wq, wk, wv, wo, start_pos):
    assert int(start_pos) == START, f"kernel hardcodes start_pos={START}"
    from concourse import bass_utils
    nc = _get_nc()
    in_maps = _host_prep(x, cache_k, cache_v, freqs_cis, mask, wq, wk, wv, wo)
    res = bass_utils.run_bass_kernel_spmd(nc, in_maps, core_ids=list(range(NCORES)))
    out = np.zeros((NT, DIM), dtype=np.float32)
    for c in range(NCORES):
        out += np.asarray(res.results[c]["out_p"], dtype=np.float32)
    return out.reshape(B, Q, DIM)
